# revision 6
# baseline (speedup 1.0000x reference)
"""CLIP attention block (LN(attn(x) @ W_out)) on 8 TRN2 NeuronCores. v3.

Problem (hardcoded): x [4, 2048, 1024] f32, mask [4, 2048] bool,
w_qkv [1024, 3072], w_out [1024, 1024], ln_g [1024].
16 heads x 64 dim, causal, scale = 1/8. Output [4, 2048, 1024] f32.

Sharding: core = (batch b, parity rho). Each core computes the final
output rows for queries of batch b with index % 2 == rho (1024 tokens);
interleaving by parity makes the causal workload identical on every core.

v3 vs v2 (331.4us): Q/K/V projections switched from fp16 matmuls to
fp8(e4m3) DoubleRow 3-term compensated matmuls: x and w are host-split
into (hi, lo) e4m3 pairs (hi = e4m3(s*t), lo = e4m3(s*t - hi)) and each
128-contraction chunk pair computes hi*hi + lo*hi + hi*lo (dropping the
~0.07% lo*lo term). DoubleRow packs 2 chunks per instruction at 0.5
cycles/row, so 8 fp16 chunk-matmuls become 12 DR matmuls at 0.75x the
PE rows with fp16-grade accuracy (e2e rel err ~2e-3 vs gate 2e-2).
Scales: x*16, w_k/w_v*32, w_q*256 (attention scale folded); descaling
is folded into the existing PSUM evacuations (1/512 into the V mask
multiplier host-side, 1/512 and 1/4096 into the K/Q copies).
fp16 attention (scores, exp, AV) and out-proj/LN unchanged from v2:
- Exact causal ranges + diagonal 0/1 mask; denominator via V aug column.
- Exp batched into 9 Act instructions/head; software-pipelined AV.
- Remaining K/Q/V projection chunks run as deadline-scheduled filler
  chains inside attention.
HW pitfalls hit: gpsimd cannot access PSUM; matmul outputs cannot cross
a PSUM bank boundary; in-place DVE reciprocal corrupts on HW (fine in
CoreSim) - use a separate output tile.
"""

import threading
from collections import deque
from contextlib import ExitStack

import numpy as np
import ml_dtypes

import concourse.bass as bass
import concourse.mybir as mybir
import concourse.tile as tile
from concourse import bacc
import concourse.bass_utils as bass_utils

F8 = mybir.dt.float8e4
F16 = mybir.dt.float16
F32 = mybir.dt.float32
E4 = ml_dtypes.float8_e4m3
DRMODE = mybir.MatmulPerfMode.DoubleRow

B, N, DIM = 4, 2048, 1024
HEADS, DH = 16, 64
INNER = HEADS * DH          # 1024
SCALE = DH ** -0.5          # 0.125
LOC = N // 2                # 1024 local query tokens per core
EPS = 1e-5

NC = 8                      # cores
HP = HEADS // 2             # 8 head pairs (128 inner dims each)
PR = 4                      # dim-chunk pairs (contraction 1024 = 4 x 256)
KC = N // 128               # 16 key chunks

XS = 16.0                   # fp8 quantization scale for x
WS = 32.0                   # fp8 scale for w_k / w_v
WQS = 256.0                 # fp8 scale for w_q (SCALE folded -> smaller w)
KDS = 1.0 / (XS * WS)       # kproj descale (psum -> KT)
QDS = 1.0 / (XS * WQS)      # qproj descale
VDS = 1.0 / (XS * WS)       # vproj descale (folded into MV host-side)

# q-range of key chunk kc: local queries [64*kc, LOC)
RNG = [LOC - 64 * kc for kc in range(KC)]


def bank_ranges(c0, c1):
    """Split [c0, c1) at 512-column boundaries (PSUM f32 bank size) —
    a matmul output may not cross a PSUM bank boundary."""
    a = c0
    while a < c1:
        b = min(c1, (a // 512 + 1) * 512)
        yield a, b
        a = b
# exp batching: groups of key chunks with total scores width <= 1024
# (kc j paired with 16-j so every pair is exactly 1024 wide)
GROUPS = [[0], [1, 15], [2, 14], [3, 13], [4, 12], [5, 11], [6, 10],
          [7, 9], [8]]


def build(reps=1):
    nc = bacc.Bacc("TRN2", target_bir_lowering=False, debug=False, num_devices=NC)

    # x^T as dim-chunk-pair tiles, e4m3 hi/lo: [pr][128][slot 2][token]
    xph = nc.dram_tensor("xph", [PR, 128, 2, N], F8, kind="ExternalInput").ap()
    xpl = nc.dram_tensor("xpl", [PR, 128, 2, N], F8, kind="ExternalInput").ap()
    xqh = nc.dram_tensor("xqh", [PR, 128, 2, LOC], F8, kind="ExternalInput").ap()
    xql = nc.dram_tensor("xql", [PR, 128, 2, LOC], F8, kind="ExternalInput").ap()
    # K/Q weights as stationary pair chunks: [hp][128][pr][slot 2][inner 128]
    wkph = nc.dram_tensor("wkph", [HP, 128, PR, 2, 128], F8, kind="ExternalInput").ap()
    wkpl = nc.dram_tensor("wkpl", [HP, 128, PR, 2, 128], F8, kind="ExternalInput").ap()
    wqph = nc.dram_tensor("wqph", [HP, 128, PR, 2, 128], F8, kind="ExternalInput").ap()
    wqpl = nc.dram_tensor("wqpl", [HP, 128, PR, 2, 128], F8, kind="ExternalInput").ap()
    # V weights as moving pair tiles: [pr][128][slot 2][inner]
    wvph = nc.dram_tensor("wvph", [PR, 128, 2, INNER], F8, kind="ExternalInput").ap()
    wvpl = nc.dram_tensor("wvpl", [PR, 128, 2, INNER], F8, kind="ExternalInput").ap()
    wout = nc.dram_tensor("wout", [INNER, DIM], F16, kind="ExternalInput").ap()
    lng = nc.dram_tensor("lng", [1, DIM], F32, kind="ExternalInput").ap()
    dmask = nc.dram_tensor("dmask", [128, 64], F16, kind="ExternalInput").ap()
    # mvecT = mask * VDS (V descale folded); mvec1 = raw mask for aug col
    mvecT = nc.dram_tensor("mvecT", [128, KC], F32, kind="ExternalInput").ap()
    mvec1 = nc.dram_tensor("mvec1", [128, KC], F32, kind="ExternalInput").ap()
    out = nc.dram_tensor("out", [LOC, DIM], F32, kind="ExternalOutput").ap()

    with nc.allow_low_precision(reason="fp8/fp16 matmul staging"), \
         tile.TileContext(nc) as tc:
        for _ in range(reps):
            _build_body(nc, tc, xph, xpl, xqh, xql, wkph, wkpl, wqph, wqpl,
                        wvph, wvpl, wout, lng, dmask, mvecT, mvec1, out)

    nc.compile()
    return nc


def _build_body(nc, tc, xph, xpl, xqh, xql, wkph, wkpl, wqph, wqpl,
                wvph, wvpl, wout, lng, dmask, mvecT, mvec1, out):
    mm = nc.tensor.matmul
    A = mybir.ActivationFunctionType
    _sc = ExitStack()

    def scope(name):
        _sc.close()
        _sc.enter_context(nc.named_scope(name))

    scope("prep")
    res = tc.alloc_tile_pool(name="res", bufs=1)
    DM = res.tile([128, 64], F16, tag="DM")
    nc.sync.dma_start(DM[:], dmask[:])
    MV = res.tile([128, KC], F32, tag="MV")
    nc.sync.dma_start(MV[:], mvecT[:])
    MV1 = res.tile([128, KC], F32, tag="MV1")
    nc.sync.dma_start(MV1[:], mvec1[:])
    ones16 = res.tile([128, HEADS, 1], F32, tag="ones16")
    nc.vector.memset(ones16[:], 1.0)
    epst = res.tile([128, 1], F32, tag="epst")
    nc.vector.memset(epst[:], EPS)
    dzero = res.tile([128, 640], F16, tag="dzero")
    nc.vector.memset(dzero[:], 0.0)

    # ---- long-lived residents (alloc order = reverse release order) ----
    resid = tc.alloc_tile_pool(name="resid", bufs=1)
    V = [resid.tile([128, HEADS, DH + 1], F16, tag=f"V{i}", name=f"V{i}")
         for i in range(KC)]
    KT = [resid.tile([128, N], F16, tag=f"kt{hp}", name=f"kt{hp}")
          for hp in range(HP)]
    QT = [resid.tile([128, LOC], F16, tag=f"qt{hp}", name=f"qt{hp}")
          for hp in range(HP)]
    OT = [resid.tile([128, LOC], F16, tag=f"otl{hp}", name=f"otl{hp}")
          for hp in range(HP)]
    WO = [resid.tile([128, DIM], F16, tag=f"wo{hp}", name=f"wo{hp}")
          for hp in range(HP)]

    # ---- input slabs (e4m3 hi/lo pair tiles) ----
    xin = tc.alloc_tile_pool(name="xin", bufs=1)
    XFH = [xin.tile([128, 2, N], F8, tag=f"xfh{pr}", name=f"xfh{pr}")
           for pr in range(PR)]
    XFL = [xin.tile([128, 2, N], F8, tag=f"xfl{pr}", name=f"xfl{pr}")
           for pr in range(PR)]
    WVH = [xin.tile([128, 2, INNER], F8, tag=f"wvh{pr}", name=f"wvh{pr}")
           for pr in range(PR)]
    WVL = [xin.tile([128, 2, INNER], F8, tag=f"wvl{pr}", name=f"wvl{pr}")
           for pr in range(PR)]
    wstr = tc.alloc_tile_pool(name="wstr", bufs=10)
    xq_pool = tc.alloc_tile_pool(name="xq", bufs=1)
    XQH = [xq_pool.tile([128, 2, LOC], F8, tag=f"xqh{pr}", name=f"xqh{pr}")
           for pr in range(PR)]
    XQL = [xq_pool.tile([128, 2, LOC], F8, tag=f"xql{pr}", name=f"xql{pr}")
           for pr in range(PR)]

    WKC = {}

    def wkc_load(hp):
        wkch = wstr.tile([128, PR, 2, 128], F8, tag="wkc", name="wkch")
        nc.gpsimd.dma_start(wkch[:], wkph[hp])
        wkcl = wstr.tile([128, PR, 2, 128], F8, tag="wkc", name="wkcl")
        nc.gpsimd.dma_start(wkcl[:], wkpl[hp])
        WKC[hp] = (wkch, wkcl)
    # V-projection inputs first (halved transfers so vproj starts early):
    # sync: XFH halves; scalar: XFL, then XQ, then WV second halves;
    # gpsimd: WV first halves, then streamed K/Q weight chunks.
    for q0, q1 in ((0, 512), (512, 1024)):
        for pr in range(PR):
            nc.sync.dma_start(XFH[pr][:, :, q0:q1], xph[pr][:, :, q0:q1])
            nc.scalar.dma_start(XFL[pr][:, :, q0:q1], xpl[pr][:, :, q0:q1])
            if q0 == 0:
                nc.gpsimd.dma_start(WVH[pr][:, :, 0:512],
                                    wvph[pr][:, :, 0:512])
                nc.gpsimd.dma_start(WVL[pr][:, :, 0:512],
                                    wvpl[pr][:, :, 0:512])
    wkc_load(0)  # hp0 K-weights right behind the WV first halves
    for pr in range(PR):
        nc.sync.dma_start(XFH[pr][:, :, N // 2:], xph[pr][:, :, N // 2:])
        nc.scalar.dma_start(XFL[pr][:, :, N // 2:], xpl[pr][:, :, N // 2:])
        nc.scalar.dma_start(XQH[pr][:], xqh[pr])
        nc.scalar.dma_start(XQL[pr][:], xql[pr])
        nc.sync.dma_start(WVH[pr][:, :, 512:], wvph[pr][:, :, 512:])
        nc.sync.dma_start(WVL[pr][:, :, 512:], wvpl[pr][:, :, 512:])

    # 3-term fp8 step list for one chunk pair: hi*hi + lo*hi + hi*lo
    def dr_steps(hi_s, lo_s, hi_m, lo_m):
        return ((hi_s, hi_m), (lo_s, hi_m), (hi_s, lo_m))

    # ---- projection chunk emitters ----
    def vproj_chunk(tci, ig, ps_pool):
        vpt = ps_pool.tile([128, 1024], F32, tag="pp", name="vp")
        vp = vpt[:, 0:512]
        ts = slice(tci * 128, (tci + 1) * 128)
        ws = slice(ig * 512, (ig + 1) * 512)
        for pr in range(PR):
            for si, (s, m) in enumerate(dr_steps(XFH[pr], XFL[pr],
                                                 WVH[pr], WVL[pr])):
                mm(vp, s[:, :, ts], m[:, :, ws],
                   start=(pr == 0 and si == 0), stop=(pr == 3 and si == 2),
                   perf_mode=DRMODE)
        nc.vector.tensor_scalar_mul(
            V[tci][:, ig * 8:(ig + 1) * 8, 0:DH],
            vp.rearrange("p (h d) -> p h d", d=DH), MV[:, tci:tci + 1])
        nc.vector.tensor_scalar_mul(
            V[tci][:, ig * 8:(ig + 1) * 8, DH:DH + 1],
            ones16[:, ig * 8:(ig + 1) * 8, :], MV1[:, tci:tci + 1])

    def kproj_chunk(hp, tg, ps_pool, copy_eng):
        kp = ps_pool.tile([128, 1024], F32, tag="pp", name="kp")
        wh, wl = WKC[hp]
        for pr in range(PR):
            for si, (s, m) in enumerate(dr_steps(wh[:, pr], wl[:, pr],
                                                 XFH[pr], XFL[pr])):
                for a, b in bank_ranges(0, 1024):
                    mm(kp[:, a:b], s, m[:, :, tg * 1024 + a:tg * 1024 + b],
                       start=(pr == 0 and si == 0),
                       stop=(pr == 3 and si == 2),
                       perf_mode=DRMODE)
        dst = KT[hp][:, tg * 1024:(tg + 1) * 1024]
        if copy_eng == "act":
            nc.scalar.activation(dst, kp[:], A.Identity, scale=KDS)
        else:
            nc.vector.tensor_scalar_mul(dst, kp[:], KDS)

    def qproj_chunk(hp, wqc, ps_pool, copy_eng):
        qp = ps_pool.tile([128, 1024], F32, tag="pp", name="qp")
        wh, wl = wqc
        for pr in range(PR):
            for si, (s, m) in enumerate(dr_steps(wh[:, pr], wl[:, pr],
                                                 XQH[pr], XQL[pr])):
                for a, b in bank_ranges(0, 1024):
                    mm(qp[:, a:b], s, m[:, :, a:b],
                       start=(pr == 0 and si == 0),
                       stop=(pr == 3 and si == 2),
                       perf_mode=DRMODE)
        if copy_eng == "act":
            nc.scalar.activation(QT[hp][:], qp[:], A.Identity, scale=QDS)
        else:
            nc.vector.tensor_scalar_mul(QT[hp][:], qp[:], QDS)

    # ---- pre-attention: vproj heads 0-7, all qproj, kproj hp0 ----
    # vproj ig=0 in two pr-major waves of 8 chains so the PE starts on
    # partial inputs and stays fed while XF/WV chunks stream in
    wave_ps = tc.alloc_tile_pool(name="waveps", bufs=8, space="PSUM")
    scope("vproj")
    # warm-up matmuls on zeros: keep the PE busy through the first input
    # DMA latency so the clock is fully ramped (p-state) when vproj starts
    dum = wave_ps.tile([128, 512], F32, tag="vp", name="dum")
    for _ in range(10):
        mm(dum[:], dzero[:, 0:128], dzero[:, 128:640],
           start=True, stop=True)
    for wv_ in range(4):
        tcis = range(wv_ * 4, wv_ * 4 + 4)
        vps = {tci: wave_ps.tile([128, 512], F32, tag="vp", name=f"vp{tci}")
               for tci in tcis}
        for pr in range(PR):
            for si, (s, m) in enumerate(dr_steps(XFH[pr], XFL[pr],
                                                 WVH[pr], WVL[pr])):
                for tci in tcis:
                    mm(vps[tci][:], s[:, :, tci * 128:(tci + 1) * 128],
                       m[:, :, 0:512],
                       start=(pr == 0 and si == 0),
                       stop=(pr == 3 and si == 2), perf_mode=DRMODE)
        for tci in tcis:
            nc.vector.tensor_scalar_mul(
                V[tci][:, 0:8, 0:DH],
                vps[tci][:].rearrange("p (h d) -> p h d", d=DH),
                MV[:, tci:tci + 1])
            nc.vector.tensor_scalar_mul(
                V[tci][:, 0:8, DH:DH + 1], ones16[:, 0:8, :],
                MV1[:, tci:tci + 1])
    # kproj0 tg0 uses the still-live wave pool (two 1-bank tiles) so it
    # doesn't wait on the wave->preps pool release barrier
    scope("kproj0")
    kpA = wave_ps.tile([128, 512], F32, tag="vp", name="kpA")
    kpB = wave_ps.tile([128, 512], F32, tag="vp", name="kpB")
    wh0, wl0 = WKC[0]
    for pr in range(PR):
        for si, (s, m) in enumerate(dr_steps(wh0[:, pr], wl0[:, pr],
                                             XFH[pr], XFL[pr])):
            mm(kpA[:], s, m[:, :, 0:512],
               start=(pr == 0 and si == 0), stop=(pr == 3 and si == 2),
               perf_mode=DRMODE)
            mm(kpB[:], s, m[:, :, 512:1024],
               start=(pr == 0 and si == 0), stop=(pr == 3 and si == 2),
               perf_mode=DRMODE)
    nc.scalar.activation(KT[0][:, 0:512], kpA[:], A.Identity, scale=KDS)
    nc.scalar.activation(KT[0][:, 512:1024], kpB[:], A.Identity, scale=KDS)
    wave_ps.release()
    pre_ps = tc.alloc_tile_pool(name="preps", bufs=3, space="PSUM")
    scope("qproj")
    wqcs = {}

    def wqc_load(pf):
        wqch = wstr.tile([128, PR, 2, 128], F8, tag="wkc", name="wqch")
        nc.gpsimd.dma_start(wqch[:], wqph[pf])
        wqcl = wstr.tile([128, PR, 2, 128], F8, tag="wkc", name="wqcl")
        nc.gpsimd.dma_start(wqcl[:], wqpl[pf])
        wqcs[pf] = (wqch, wqcl)

    for pf in (0, 1, 2):
        wqc_load(pf)
    qproj_chunk(0, wqcs.pop(0), pre_ps, "act")
    scope("kproj0b")
    kproj_chunk(0, 1, pre_ps, "act")

    for hp in range(HP):
        (nc.sync if hp % 2 == 0 else nc.scalar).dma_start(
            WO[hp][:], wout[hp * 128:(hp + 1) * 128, :])

    # ---- attention (+ interleaved remaining projections) ----
    pre_ps.release()
    proj_ps = tc.alloc_tile_pool(name="projps", bufs=1, space="PSUM")
    st_ps = tc.alloc_tile_pool(name="stps", bufs=2, space="PSUM")
    o_ps = tc.alloc_tile_pool(name="ops", bufs=1, space="PSUM")
    pt_pool = tc.alloc_tile_pool(name="pt", bufs=3)
    nrm_pool = tc.alloc_tile_pool(name="nrm", bufs=1)

    # filler chains, deadline-interleaved: kproj(hp) must land well before
    # attention head 2*hp; vproj heads 8-15 before head 8. One chain fires
    # every 3rd pipeline slot (9 slots per head).
    wkc_load(1)

    def kf(hp, tg):
        def f():
            kproj_chunk(hp, tg, proj_ps, "dve")
            if tg == 1:
                if hp + 1 < HP:
                    wkc_load(hp + 1)
                WKC.pop(hp)
        return f

    def vf(tci):
        return lambda: vproj_chunk(tci, 1, proj_ps)

    def qf(hp):
        def f():
            if hp + 2 < HP:
                wqc_load(hp + 2)
            qproj_chunk(hp, wqcs.pop(hp), proj_ps, "dve")
        return f

    FILL = deque([
        qf(1), kf(1, 0), kf(1, 1), vf(0), vf(1),
        qf(2), kf(2, 0), kf(2, 1), vf(2), vf(3),
        qf(3), kf(3, 0), kf(3, 1), vf(4), vf(5),
        qf(4), kf(4, 0), kf(4, 1), vf(6), vf(7),
        vf(8), vf(9), vf(10), vf(11), vf(12), vf(13), vf(14), vf(15),
        qf(5), kf(5, 0), kf(5, 1), qf(6), kf(6, 0), kf(6, 1),
        qf(7), kf(7, 0), kf(7, 1),
    ])

    def fill(n=1):
        for _ in range(n):
            if FILL:
                with nc.named_scope("fillp"):
                    FILL.popleft()()

    _slot = [0, 2, 0]

    def fill_slot():
        # spread filler chains over the attention span: fast enough early
        # that every chain beats its consumer head (~1 per 2.5 slots),
        # stretched to 1 per 5 for the last chains so late Act-bound heads
        # still have PE work
        _slot[0] += 1
        if _slot[0] >= _slot[1]:
            fill(1)
            if len(FILL) > 9:
                _slot[2] ^= 1
                _slot[1] += 2 + _slot[2]
            else:
                _slot[1] += 5

    scope("attn")
    deferred_norm = deque()
    for hp in range(HP):
        for h2 in range(2):
            h = 2 * hp + h2
            hs = slice(h2 * DH, (h2 + 1) * DH)
            ot = o_ps.tile([DH + 1, LOC], F32, tag="o", name="ot")

            def emit_av(p, ot=ot, h=h):
                grp, offs, pt, last = p
                for gi, (kc, off) in enumerate(zip(grp, offs)):
                    pieces = list(bank_ranges(64 * kc, LOC))
                    for pi, (a, b) in enumerate(pieces):
                        mm(ot[:, a:b], V[kc][:, h, :],
                           pt[:, off + a - 64 * kc:off + b - 64 * kc],
                           start=(kc == 0),
                           stop=(last and gi == len(grp) - 1
                                 and pi == len(pieces) - 1),
                           skip_group_check=True)

            pend = deque()  # software pipeline, depth 2
            for gi, grp in enumerate(GROUPS):
                W = sum(RNG[kc] for kc in grp)
                st = st_ps.tile([128, W], F32, tag="st", name="st")
                offs = []
                off = 0
                for kc in grp:
                    for a, b in bank_ranges(off, off + RNG[kc]):
                        mm(st[:, a:b], KT[hp][hs, kc * 128:(kc + 1) * 128],
                           QT[hp][hs, 64 * kc + a - off:64 * kc + b - off],
                           start=True, stop=True)
                    offs.append(off)
                    off += RNG[kc]
                pt = pt_pool.tile([128, W], F16, tag="pt", name="pt")
                nc.scalar.activation(pt[:], st[:], A.Exp)
                for kc, off in zip(grp, offs):  # diagonal 0/1 mask
                    nc.vector.tensor_mul(pt[:, off:off + 64],
                                         pt[:, off:off + 64], DM[:])
                if gi == 0 and deferred_norm:
                    deferred_norm.popleft()()
                pend.append((grp, offs, pt, grp is GROUPS[-1]))
                if len(pend) > 2:
                    emit_av(pend.popleft())
                fill_slot()
            while pend:
                emit_av(pend.popleft())
                fill_slot()

            # normalize: evacuate O^T from PSUM immediately (frees the bank
            # for the next head's AV); defer the reciprocal/broadcast/scale
            # past the next head's first mask-muls so they don't block its
            # AV start on the DVE queue. fp16 throughout: numerator/denom
            # magnitudes stay well inside fp16 range and the 2-byte packed
            # SBUF operands hit the DVE 2x/4x fast paths
            ocp = nrm_pool.tile([DH + 1, LOC], F16, tag="ocp", name="ocp")
            nc.vector.tensor_copy(ocp[:], ot[:])

            def norm_rest(ocp=ocp, hp=hp, hs=hs):
                rcp = nrm_pool.tile([1, LOC], F16, tag="rcp", name="rcp")
                nc.vector.reciprocal(rcp[:], ocp[DH:DH + 1, :])
                rbs = nrm_pool.tile([DH, LOC], F16, tag="rbs", name="rbs")
                nc.gpsimd.partition_broadcast(rbs[:], rcp[:])
                nc.vector.tensor_mul(OT[hp][hs, :], ocp[0:DH, :], rbs[:])

            deferred_norm.append(norm_rest)

    while deferred_norm:
        deferred_norm.popleft()()
    while FILL:
        fill(1)

    # ---- out projection + layernorm ----
    scope("outln")
    nrm_pool.release()
    pt_pool.release()
    o_ps.release()
    st_ps.release()
    proj_ps.release()
    xq_pool.release()
    wstr.release()
    xin.release()

    gz = tc.alloc_tile_pool(name="gz", bufs=1)
    grow = gz.tile([1, DIM], F32, tag="grow")
    nc.sync.dma_start(grow[:], lng[:])
    GB = gz.tile([128, DIM], F32, tag="GB")
    nc.gpsimd.partition_broadcast(GB[:], grow[:])
    z_ps = tc.alloc_tile_pool(name="zps", bufs=1, space="PSUM")
    stat = tc.alloc_tile_pool(name="stat", bufs=2)
    stage = tc.alloc_tile_pool(name="stage", bufs=2)

    # bank-panel pipeline: each token-chunk tb runs as two 512-col panel
    # chains; bank-0 stats (sum + sum-of-squares) run on DVE/ACT while the
    # PE fills bank 1, so the post-matmul LN tail of the LAST chunk is just
    # bank-1 stats + combine + normalize instead of a full-row chain
    for tb in range(8):
        zb = []
        ss = []
        qq = []
        for bk in range(2):
            zt = z_ps.tile([128, 512], F32, tag=f"z{(2 * tb + bk) % 4}",
                           name=f"z{tb}_{bk}")
            a = bk * 512
            for hp in range(HP):
                mm(zt[:], OT[hp][:, tb * 128:(tb + 1) * 128],
                   WO[hp][:, a:a + 512], start=(hp == 0), stop=(hp == HP - 1))
            s_ = stat.tile([128, 1], F32, tag=f"s{bk}", name="s")
            nc.vector.reduce_sum(s_[:], zt[:], axis=mybir.AxisListType.X)
            q_ = stat.tile([128, 1], F32, tag=f"q{bk}", name="q")
            scr = stage.tile([128, 512], F32, tag=f"scr{bk}", name="scr")
            nc.scalar.activation(scr[:], zt[:], A.Square, accum_out=q_[:])
            zb.append(zt)
            ss.append(s_)
            qq.append(q_)
        mean_n = stat.tile([128, 1], F32, tag="mean", name="mean")
        nc.vector.tensor_add(mean_n[:], ss[0][:], ss[1][:])
        nc.vector.tensor_scalar_mul(mean_n[:], mean_n[:], -1.0 / DIM)
        msq = stat.tile([128, 1], F32, tag="msq", name="msq")
        nc.vector.tensor_add(msq[:], qq[0][:], qq[1][:])
        nc.vector.tensor_scalar_mul(msq[:], msq[:], 1.0 / DIM)
        var = stat.tile([128, 1], F32, tag="var", name="var")
        nc.vector.tensor_mul(var[:], mean_n[:], mean_n[:])
        nc.vector.tensor_sub(var[:], msq[:], var[:])
        std = stat.tile([128, 1], F32, tag="std", name="std")
        nc.scalar.activation(std[:], var[:], A.Sqrt, bias=epst[:])
        rstd = stat.tile([128, 1], F32, tag="rstd", name="rstd")
        nc.vector.reciprocal(rstd[:], std[:])
        nmr = stat.tile([128, 1], F32, tag="nmr", name="nmr")
        nc.vector.tensor_mul(nmr[:], mean_n[:], rstd[:])
        zn = stage.tile([128, DIM], F32, tag="zn", name="zn")
        outb = stage.tile([128, DIM], F32, tag="outb", name="outb")
        for bk in range(2):  # halves pipeline zn->mul->DMA
            a = bk * 512
            nc.scalar.activation(zn[:, a:a + 512], zb[bk][:], A.Identity,
                                 bias=nmr[:], scale=rstd[:])
            nc.vector.tensor_mul(outb[:, a:a + 512], zn[:, a:a + 512],
                                 GB[:, a:a + 512])
            (nc.sync if tb % 2 == 0 else nc.gpsimd).dma_start(
                out[tb * 128:(tb + 1) * 128, a:a + 512], outb[:, a:a + 512])

    _sc.close()
    stage.release()
    stat.release()
    z_ps.release()
    gz.release()
    resid.release()
    res.release()


def _split8(t):
    """fp32 array -> (hi, lo) e4m3 pair."""
    hi = t.astype(E4)
    lo = (t - hi.astype(np.float32)).astype(E4)
    return hi, lo


def _xpairs(xT):
    """[DIM, n] f32 -> [PR, 128, 2, n] pair layout (chunk 2pr+s rows)."""
    n = xT.shape[1]
    return np.ascontiguousarray(xT.reshape(PR, 2, 128, n).transpose(0, 2, 1, 3))


def _wpairs(w):
    """[DIM, INNER] f32 -> [HP, 128, PR, 2, 128]:
    wb[hp, p, pr, s, j] = w[(2*pr+s)*128 + p, hp*128 + j]."""
    return np.ascontiguousarray(
        w.reshape(PR, 2, 128, HP, 128).transpose(3, 2, 0, 1, 4))


def make_in_maps(x, mask, w_qkv, w_out, ln_g):
    x = np.asarray(x, dtype=np.float32)
    mask_np = np.asarray(mask)
    w_qkv = np.asarray(w_qkv, dtype=np.float32)

    wq_s = w_qkv[:, :INNER] * (SCALE * WQS)
    wk_s = w_qkv[:, INNER:2 * INNER] * WS
    wv_s = w_qkv[:, 2 * INNER:] * WS
    wqh_, wql_ = _split8(wq_s)
    wkh_, wkl_ = _split8(wk_s)
    wvh_, wvl_ = _split8(wv_s)
    wqph = _wpairs(wqh_.astype(np.float32)).astype(E4)
    wqpl = _wpairs(wql_.astype(np.float32)).astype(E4)
    wkph = _wpairs(wkh_.astype(np.float32)).astype(E4)
    wkpl = _wpairs(wkl_.astype(np.float32)).astype(E4)
    # V weights: moving layout [PR, 128, 2, INNER]
    wvph = _xpairs(wvh_.astype(np.float32)).astype(E4)
    wvpl = _xpairs(wvl_.astype(np.float32)).astype(E4)
    wout16 = np.ascontiguousarray(np.asarray(w_out, np.float32)).astype(np.float16)
    lng = np.ascontiguousarray(np.asarray(ln_g, np.float32)[None, :])

    # diagonal 0/1 mask [128 keys, 64 queries]: key 128*kc+kk is causally
    # valid for local query 64*kc+m (global 2*(64*kc+m)+rho) iff kk <= 2m+rho
    kk = np.arange(128)[:, None]
    mq = np.arange(64)[None, :]
    dmasks = {rho: np.ascontiguousarray((kk <= 2 * mq + rho).astype(np.float16))
              for rho in (0, 1)}

    in_maps = []
    for b in range(B):
        xT = np.ascontiguousarray(x[b].T) * XS  # [DIM, N] scaled
        xh_, xl_ = _split8(xT)
        xph = _xpairs(xh_.astype(np.float32)).astype(E4)
        xpl = _xpairs(xl_.astype(np.float32)).astype(E4)
        mv = mask_np[b].astype(np.float32)  # [N]
        mvecT = np.ascontiguousarray(mv.reshape(KC, 128).T) * VDS  # [128, KC]
        mvec1 = np.ascontiguousarray(mv.reshape(KC, 128).T)
        for rho in (0, 1):
            xqT = np.ascontiguousarray(x[b].T[:, rho::2]) * XS
            xqh_, xql_ = _split8(xqT)
            xqh = _xpairs(xqh_.astype(np.float32)).astype(E4)
            xql = _xpairs(xql_.astype(np.float32)).astype(E4)
            in_maps.append({
                "xph": xph, "xpl": xpl, "xqh": xqh, "xql": xql,
                "wkph": wkph, "wkpl": wkpl, "wqph": wqph, "wqpl": wqpl,
                "wvph": wvph, "wvpl": wvpl,
                "wout": wout16, "lng": lng, "dmask": dmasks[rho],
                "mvecT": mvecT, "mvec1": mvec1,
            })
    return in_maps


_CACHE = {}
_LOCK = threading.Lock()
RUN_KWARGS = {}


def _get_nc():
    with _LOCK:
        if "nc" not in _CACHE:
            _CACHE["nc"] = build()
    return _CACHE["nc"]


def kernel(x, mask, w_qkv, w_out, ln_g):
    in_maps = make_in_maps(x, mask, w_qkv, w_out, ln_g)
    nc = _get_nc()
    res = bass_utils.run_bass_kernel_spmd(nc, in_maps, core_ids=list(range(NC)),
                                          **RUN_KWARGS)
    _CACHE["last"] = res

    final = np.empty((B, N, DIM), dtype=np.float32)
    for b in range(B):
        for rho in (0, 1):
            final[b, rho::2, :] = res.results[2 * b + rho]["out"]
    return final


# revision 7
# speedup vs baseline: 1.0380x; 1.0380x over previous
"""CLIP attention block (LN(attn(x) @ W_out)) on 8 TRN2 NeuronCores. v3.

Problem (hardcoded): x [4, 2048, 1024] f32, mask [4, 2048] bool,
w_qkv [1024, 3072], w_out [1024, 1024], ln_g [1024].
16 heads x 64 dim, causal, scale = 1/8. Output [4, 2048, 1024] f32.

Sharding: core = (batch b, parity rho). Each core computes the final
output rows for queries of batch b with index % 2 == rho (1024 tokens);
interleaving by parity makes the causal workload identical on every core.

v3 vs v2 (331.4us): Q/K/V projections switched from fp16 matmuls to
fp8(e4m3) DoubleRow 3-term compensated matmuls: x and w are host-split
into (hi, lo) e4m3 pairs (hi = e4m3(s*t), lo = e4m3(s*t - hi)) and each
128-contraction chunk pair computes hi*hi + lo*hi + hi*lo (dropping the
~0.07% lo*lo term). DoubleRow packs 2 chunks per instruction at 0.5
cycles/row, so 8 fp16 chunk-matmuls become 12 DR matmuls at 0.75x the
PE rows with fp16-grade accuracy (e2e rel err ~2e-3 vs gate 2e-2).
Scales: x*16, w_k/w_v*32, w_q*256 (attention scale folded); descaling
is folded into the existing PSUM evacuations (1/512 into the V mask
multiplier host-side, 1/512 and 1/4096 into the K/Q copies).
fp16 attention (scores, exp, AV) and out-proj/LN unchanged from v2:
- Exact causal ranges + diagonal 0/1 mask; denominator via V aug column.
- Exp batched into 9 Act instructions/head; software-pipelined AV.
- Remaining K/Q/V projection chunks run as deadline-scheduled filler
  chains inside attention.
HW pitfalls hit: gpsimd cannot access PSUM; matmul outputs cannot cross
a PSUM bank boundary; in-place DVE reciprocal corrupts on HW (fine in
CoreSim) - use a separate output tile.
"""

import threading
from collections import deque
from contextlib import ExitStack

import numpy as np
import ml_dtypes

import concourse.bass as bass
import concourse.mybir as mybir
import concourse.tile as tile
from concourse import bacc
import concourse.bass_utils as bass_utils

F8 = mybir.dt.float8e4
F16 = mybir.dt.float16
F32 = mybir.dt.float32
E4 = ml_dtypes.float8_e4m3
DRMODE = mybir.MatmulPerfMode.DoubleRow

B, N, DIM = 4, 2048, 1024
HEADS, DH = 16, 64
INNER = HEADS * DH          # 1024
SCALE = DH ** -0.5          # 0.125
LOC = N // 2                # 1024 local query tokens per core
EPS = 1e-5

NC = 8                      # cores
HP = HEADS // 2             # 8 head pairs (128 inner dims each)
PR = 4                      # dim-chunk pairs (contraction 1024 = 4 x 256)
KC = N // 128               # 16 key chunks

XS = 16.0                   # fp8 quantization scale for x
WS = 32.0                   # fp8 scale for w_k / w_v
WQS = 256.0                 # fp8 scale for w_q (SCALE folded -> smaller w)
KDS = 1.0 / (XS * WS)       # kproj descale (psum -> KT)
QDS = 1.0 / (XS * WQS)      # qproj descale
VDS = 1.0 / (XS * WS)       # vproj descale (folded into MV host-side)

# q-range of key chunk kc: local queries [64*kc, LOC)
RNG = [LOC - 64 * kc for kc in range(KC)]


def bank_ranges(c0, c1):
    """Split [c0, c1) at 512-column boundaries (PSUM f32 bank size) —
    a matmul output may not cross a PSUM bank boundary."""
    a = c0
    while a < c1:
        b = min(c1, (a // 512 + 1) * 512)
        yield a, b
        a = b
# exp batching: groups of key chunks with total scores width <= 1024
# (kc j paired with 16-j so every pair is exactly 1024 wide)
GROUPS = [[0], [1, 15], [2, 14], [3, 13], [4, 12], [5, 11], [6, 10],
          [7, 9], [8]]


def build(reps=1):
    nc = bacc.Bacc("TRN2", target_bir_lowering=False, debug=False, num_devices=NC)

    # x^T as dim-chunk-pair tiles, e4m3 hi/lo: [pr][128][slot 2][token]
    xph = nc.dram_tensor("xph", [PR, 128, 2, N], F8, kind="ExternalInput").ap()
    xpl = nc.dram_tensor("xpl", [PR, 128, 2, N], F8, kind="ExternalInput").ap()
    xqh = nc.dram_tensor("xqh", [PR, 128, 2, LOC], F8, kind="ExternalInput").ap()
    xql = nc.dram_tensor("xql", [PR, 128, 2, LOC], F8, kind="ExternalInput").ap()
    # K/Q weights as stationary pair chunks: [hp][128][pr][slot 2][inner 128]
    wkph = nc.dram_tensor("wkph", [HP, 128, PR, 2, 128], F8, kind="ExternalInput").ap()
    wkpl = nc.dram_tensor("wkpl", [HP, 128, PR, 2, 128], F8, kind="ExternalInput").ap()
    wqph = nc.dram_tensor("wqph", [HP, 128, PR, 2, 128], F8, kind="ExternalInput").ap()
    wqpl = nc.dram_tensor("wqpl", [HP, 128, PR, 2, 128], F8, kind="ExternalInput").ap()
    # V weights as moving pair tiles: [pr][128][slot 2][inner]
    wvph = nc.dram_tensor("wvph", [PR, 128, 2, INNER], F8, kind="ExternalInput").ap()
    wvpl = nc.dram_tensor("wvpl", [PR, 128, 2, INNER], F8, kind="ExternalInput").ap()
    wout = nc.dram_tensor("wout", [INNER, DIM], F16, kind="ExternalInput").ap()
    lng = nc.dram_tensor("lng", [1, DIM], F32, kind="ExternalInput").ap()
    dmask = nc.dram_tensor("dmask", [128, 64], F16, kind="ExternalInput").ap()
    # mvecT = mask * VDS (V descale folded); mvec1 = raw mask for aug col
    mvecT = nc.dram_tensor("mvecT", [128, KC], F32, kind="ExternalInput").ap()
    mvec1 = nc.dram_tensor("mvec1", [128, KC], F32, kind="ExternalInput").ap()
    out = nc.dram_tensor("out", [LOC, DIM], F32, kind="ExternalOutput").ap()

    with nc.allow_low_precision(reason="fp8/fp16 matmul staging"), \
         tile.TileContext(nc) as tc:
        for _ in range(reps):
            _build_body(nc, tc, xph, xpl, xqh, xql, wkph, wkpl, wqph, wqpl,
                        wvph, wvpl, wout, lng, dmask, mvecT, mvec1, out)

    nc.compile()
    return nc


def _build_body(nc, tc, xph, xpl, xqh, xql, wkph, wkpl, wqph, wqpl,
                wvph, wvpl, wout, lng, dmask, mvecT, mvec1, out):
    mm = nc.tensor.matmul
    A = mybir.ActivationFunctionType
    _sc = ExitStack()

    def scope(name):
        _sc.close()
        _sc.enter_context(nc.named_scope(name))

    scope("prep")
    res = tc.alloc_tile_pool(name="res", bufs=1)
    DM = res.tile([128, 64], F16, tag="DM")
    nc.sync.dma_start(DM[:], dmask[:])
    MV = res.tile([128, KC], F32, tag="MV")
    nc.sync.dma_start(MV[:], mvecT[:])
    MV1 = res.tile([128, KC], F32, tag="MV1")
    nc.sync.dma_start(MV1[:], mvec1[:])
    ones16 = res.tile([128, HEADS, 1], F32, tag="ones16")
    nc.vector.memset(ones16[:], 1.0)
    epst = res.tile([128, 1], F32, tag="epst")
    nc.vector.memset(epst[:], EPS)
    dzero = res.tile([128, 640], F16, tag="dzero")
    nc.vector.memset(dzero[:], 0.0)

    # ---- long-lived residents (alloc order = reverse release order) ----
    resid = tc.alloc_tile_pool(name="resid", bufs=1)
    V = [resid.tile([128, HEADS, DH + 1], F16, tag=f"V{i}", name=f"V{i}")
         for i in range(KC)]
    KT = [resid.tile([128, N], F16, tag=f"kt{hp}", name=f"kt{hp}")
          for hp in range(HP)]
    QT = [resid.tile([128, LOC], F16, tag=f"qt{hp}", name=f"qt{hp}")
          for hp in range(HP)]
    OT = [resid.tile([128, LOC], F16, tag=f"otl{hp}", name=f"otl{hp}")
          for hp in range(HP)]
    WO = [resid.tile([128, DIM], F16, tag=f"wo{hp}", name=f"wo{hp}")
          for hp in range(HP)]

    # ---- input slabs (e4m3 hi/lo pair tiles) ----
    xin = tc.alloc_tile_pool(name="xin", bufs=1)
    XFH = [xin.tile([128, 2, N], F8, tag=f"xfh{pr}", name=f"xfh{pr}")
           for pr in range(PR)]
    XFL = [xin.tile([128, 2, N], F8, tag=f"xfl{pr}", name=f"xfl{pr}")
           for pr in range(PR)]
    WVH = [xin.tile([128, 2, INNER], F8, tag=f"wvh{pr}", name=f"wvh{pr}")
           for pr in range(PR)]
    WVL = [xin.tile([128, 2, INNER], F8, tag=f"wvl{pr}", name=f"wvl{pr}")
           for pr in range(PR)]
    wstr = tc.alloc_tile_pool(name="wstr", bufs=10)
    xq_pool = tc.alloc_tile_pool(name="xq", bufs=1)
    XQH = [xq_pool.tile([128, 2, LOC], F8, tag=f"xqh{pr}", name=f"xqh{pr}")
           for pr in range(PR)]
    XQL = [xq_pool.tile([128, 2, LOC], F8, tag=f"xql{pr}", name=f"xql{pr}")
           for pr in range(PR)]

    WKC = {}

    def wkc_load(hp):
        wkch = wstr.tile([128, PR, 2, 128], F8, tag="wkc", name="wkch")
        nc.gpsimd.dma_start(wkch[:], wkph[hp])
        wkcl = wstr.tile([128, PR, 2, 128], F8, tag="wkc", name="wkcl")
        nc.gpsimd.dma_start(wkcl[:], wkpl[hp])
        WKC[hp] = (wkch, wkcl)
    # V-projection inputs first (halved transfers so vproj starts early):
    # sync: XFH halves; scalar: XFL, then XQ, then WV second halves;
    # gpsimd: WV first halves, then streamed K/Q weight chunks.
    for q0, q1 in ((0, 512), (512, 1024)):
        for pr in range(PR):
            nc.sync.dma_start(XFH[pr][:, :, q0:q1], xph[pr][:, :, q0:q1])
            nc.scalar.dma_start(XFL[pr][:, :, q0:q1], xpl[pr][:, :, q0:q1])
            if q0 == 0:
                nc.gpsimd.dma_start(WVH[pr][:, :, 0:512],
                                    wvph[pr][:, :, 0:512])
                nc.gpsimd.dma_start(WVL[pr][:, :, 0:512],
                                    wvpl[pr][:, :, 0:512])
    wkc_load(0)  # hp0 K-weights right behind the WV first halves
    for pr in range(PR):
        nc.sync.dma_start(XFH[pr][:, :, N // 2:], xph[pr][:, :, N // 2:])
        nc.scalar.dma_start(XFL[pr][:, :, N // 2:], xpl[pr][:, :, N // 2:])
        nc.scalar.dma_start(XQH[pr][:], xqh[pr])
        nc.scalar.dma_start(XQL[pr][:], xql[pr])
        nc.sync.dma_start(WVH[pr][:, :, 512:], wvph[pr][:, :, 512:])
        nc.sync.dma_start(WVL[pr][:, :, 512:], wvpl[pr][:, :, 512:])

    # 3-term fp8 step list for one chunk pair: hi*hi + lo*hi + hi*lo
    def dr_steps(hi_s, lo_s, hi_m, lo_m):
        return ((hi_s, hi_m), (lo_s, hi_m), (hi_s, lo_m))

    # ---- projection chunk emitters ----
    def vproj_chunk(tci, ig, ps_pool):
        vpt = ps_pool.tile([128, 1024], F32, tag="pp", name="vp")
        vp = vpt[:, 0:512]
        ts = slice(tci * 128, (tci + 1) * 128)
        ws = slice(ig * 512, (ig + 1) * 512)
        for pr in range(PR):
            for si, (s, m) in enumerate(dr_steps(XFH[pr], XFL[pr],
                                                 WVH[pr], WVL[pr])):
                mm(vp, s[:, :, ts], m[:, :, ws],
                   start=(pr == 0 and si == 0), stop=(pr == 3 and si == 2),
                   perf_mode=DRMODE)
        nc.vector.tensor_scalar_mul(
            V[tci][:, ig * 8:(ig + 1) * 8, 0:DH],
            vp.rearrange("p (h d) -> p h d", d=DH), MV[:, tci:tci + 1])
        nc.vector.tensor_scalar_mul(
            V[tci][:, ig * 8:(ig + 1) * 8, DH:DH + 1],
            ones16[:, ig * 8:(ig + 1) * 8, :], MV1[:, tci:tci + 1])

    def kproj_chunk(hp, tg, ps_pool, copy_eng):
        kp = ps_pool.tile([128, 1024], F32, tag="pp", name="kp")
        wh, wl = WKC[hp]
        for pr in range(PR):
            for si, (s, m) in enumerate(dr_steps(wh[:, pr], wl[:, pr],
                                                 XFH[pr], XFL[pr])):
                for a, b in bank_ranges(0, 1024):
                    mm(kp[:, a:b], s, m[:, :, tg * 1024 + a:tg * 1024 + b],
                       start=(pr == 0 and si == 0),
                       stop=(pr == 3 and si == 2),
                       perf_mode=DRMODE)
        dst = KT[hp][:, tg * 1024:(tg + 1) * 1024]
        if copy_eng == "act":
            nc.scalar.activation(dst, kp[:], A.Identity, scale=KDS)
        else:
            nc.vector.tensor_scalar_mul(dst, kp[:], KDS)

    def qproj_chunk(hp, wqc, ps_pool, copy_eng):
        qp = ps_pool.tile([128, 1024], F32, tag="pp", name="qp")
        wh, wl = wqc
        for pr in range(PR):
            for si, (s, m) in enumerate(dr_steps(wh[:, pr], wl[:, pr],
                                                 XQH[pr], XQL[pr])):
                for a, b in bank_ranges(0, 1024):
                    mm(qp[:, a:b], s, m[:, :, a:b],
                       start=(pr == 0 and si == 0),
                       stop=(pr == 3 and si == 2),
                       perf_mode=DRMODE)
        if copy_eng == "act":
            nc.scalar.activation(QT[hp][:], qp[:], A.Identity, scale=QDS)
        else:
            nc.vector.tensor_scalar_mul(QT[hp][:], qp[:], QDS)

    # ---- pre-attention: vproj heads 0-7, all qproj, kproj hp0 ----
    # vproj ig=0 in two pr-major waves of 8 chains so the PE starts on
    # partial inputs and stays fed while XF/WV chunks stream in
    wave_ps = tc.alloc_tile_pool(name="waveps", bufs=8, space="PSUM")
    scope("vproj")
    # warm-up matmuls on zeros: keep the PE busy through the first input
    # DMA latency so the clock is fully ramped (p-state) when vproj starts
    dum = wave_ps.tile([128, 512], F32, tag="vp", name="dum")
    for _ in range(10):
        mm(dum[:], dzero[:, 0:128], dzero[:, 128:640],
           start=True, stop=True)
    for wv_ in range(4):
        tcis = range(wv_ * 4, wv_ * 4 + 4)
        vps = {tci: wave_ps.tile([128, 512], F32, tag="vp", name=f"vp{tci}")
               for tci in tcis}
        for pr in range(PR):
            for si, (s, m) in enumerate(dr_steps(XFH[pr], XFL[pr],
                                                 WVH[pr], WVL[pr])):
                for tci in tcis:
                    mm(vps[tci][:], s[:, :, tci * 128:(tci + 1) * 128],
                       m[:, :, 0:512],
                       start=(pr == 0 and si == 0),
                       stop=(pr == 3 and si == 2), perf_mode=DRMODE)
        for tci in tcis:
            nc.vector.tensor_scalar_mul(
                V[tci][:, 0:8, 0:DH],
                vps[tci][:].rearrange("p (h d) -> p h d", d=DH),
                MV[:, tci:tci + 1])
            nc.vector.tensor_scalar_mul(
                V[tci][:, 0:8, DH:DH + 1], ones16[:, 0:8, :],
                MV1[:, tci:tci + 1])
    # kproj0 tg0 uses the still-live wave pool (two 1-bank tiles) so it
    # doesn't wait on the wave->preps pool release barrier
    scope("kproj0")
    kpA = wave_ps.tile([128, 512], F32, tag="vp", name="kpA")
    kpB = wave_ps.tile([128, 512], F32, tag="vp", name="kpB")
    wh0, wl0 = WKC[0]
    for pr in range(PR):
        for si, (s, m) in enumerate(dr_steps(wh0[:, pr], wl0[:, pr],
                                             XFH[pr], XFL[pr])):
            mm(kpA[:], s, m[:, :, 0:512],
               start=(pr == 0 and si == 0), stop=(pr == 3 and si == 2),
               perf_mode=DRMODE)
            mm(kpB[:], s, m[:, :, 512:1024],
               start=(pr == 0 and si == 0), stop=(pr == 3 and si == 2),
               perf_mode=DRMODE)
    nc.scalar.activation(KT[0][:, 0:512], kpA[:], A.Identity, scale=KDS)
    nc.scalar.activation(KT[0][:, 512:1024], kpB[:], A.Identity, scale=KDS)
    wave_ps.release()
    pre_ps = tc.alloc_tile_pool(name="preps", bufs=3, space="PSUM")
    scope("qproj")
    wqcs = {}

    def wqc_load(pf):
        wqch = wstr.tile([128, PR, 2, 128], F8, tag="wkc", name="wqch")
        nc.gpsimd.dma_start(wqch[:], wqph[pf])
        wqcl = wstr.tile([128, PR, 2, 128], F8, tag="wkc", name="wqcl")
        nc.gpsimd.dma_start(wqcl[:], wqpl[pf])
        wqcs[pf] = (wqch, wqcl)

    for pf in (0, 1, 2):
        wqc_load(pf)
    qproj_chunk(0, wqcs.pop(0), pre_ps, "act")
    scope("kproj0b")
    kproj_chunk(0, 1, pre_ps, "act")

    for hp in range(HP):
        (nc.sync if hp % 2 == 0 else nc.scalar).dma_start(
            WO[hp][:], wout[hp * 128:(hp + 1) * 128, :])

    # ---- attention (+ interleaved remaining projections) ----
    pre_ps.release()
    proj_ps = tc.alloc_tile_pool(name="projps", bufs=1, space="PSUM")
    st_ps = tc.alloc_tile_pool(name="stps", bufs=2, space="PSUM")
    o_ps = tc.alloc_tile_pool(name="ops", bufs=1, space="PSUM")
    pt_pool = tc.alloc_tile_pool(name="pt", bufs=3)
    nrm_pool = tc.alloc_tile_pool(name="nrm", bufs=1)

    # filler chains, deadline-interleaved: kproj(hp) must land well before
    # attention head 2*hp; vproj heads 8-15 before head 8. One chain fires
    # every 3rd pipeline slot (9 slots per head).
    wkc_load(1)

    def kf(hp, tg):
        def f():
            kproj_chunk(hp, tg, proj_ps, "dve")
            if tg == 1:
                if hp + 1 < HP:
                    wkc_load(hp + 1)
                WKC.pop(hp)
        return f

    def vf(tci):
        return lambda: vproj_chunk(tci, 1, proj_ps)

    def qf(hp):
        def f():
            if hp + 2 < HP:
                wqc_load(hp + 2)
            qproj_chunk(hp, wqcs.pop(hp), proj_ps, "dve")
        return f

    FILL = deque([
        qf(1), kf(1, 0), kf(1, 1), vf(0), vf(1),
        qf(2), kf(2, 0), kf(2, 1), vf(2), vf(3),
        qf(3), kf(3, 0), kf(3, 1), vf(4), vf(5),
        qf(4), kf(4, 0), kf(4, 1), vf(6), vf(7),
        vf(8), vf(9), vf(10), vf(11), vf(12), vf(13), vf(14), vf(15),
        qf(5), kf(5, 0), kf(5, 1), qf(6), kf(6, 0), kf(6, 1),
        qf(7), kf(7, 0), kf(7, 1),
    ])

    def fill(n=1):
        for _ in range(n):
            if FILL:
                with nc.named_scope("fillp"):
                    FILL.popleft()()

    _slot = [0, 2, 0]

    def fill_slot():
        # spread filler chains over the attention span: fast enough early
        # that every chain beats its consumer head (~1 per 2.5 slots),
        # stretched to 1 per 5 for the last chains so late Act-bound heads
        # still have PE work
        _slot[0] += 1
        if _slot[0] >= _slot[1]:
            fill(1)
            if len(FILL) > 9:
                _slot[2] ^= 1
                _slot[1] += 2 + _slot[2]
            else:
                _slot[1] += 5

    scope("attn")
    deferred_norm = deque()
    for hp in range(HP):
        for h2 in range(2):
            h = 2 * hp + h2
            hs = slice(h2 * DH, (h2 + 1) * DH)
            ot = o_ps.tile([DH + 1, LOC], F32, tag="o", name="ot")

            def emit_av(p, ot=ot, h=h):
                grp, offs, pt, last = p
                for gi, (kc, off) in enumerate(zip(grp, offs)):
                    pieces = list(bank_ranges(64 * kc, LOC))
                    for pi, (a, b) in enumerate(pieces):
                        mm(ot[:, a:b], V[kc][:, h, :],
                           pt[:, off + a - 64 * kc:off + b - 64 * kc],
                           start=(kc == 0),
                           stop=(last and gi == len(grp) - 1
                                 and pi == len(pieces) - 1),
                           skip_group_check=True)

            pend = deque()  # software pipeline, depth 2
            for gi, grp in enumerate(GROUPS):
                W = sum(RNG[kc] for kc in grp)
                st = st_ps.tile([128, W], F32, tag="st", name="st")
                offs = []
                off = 0
                for kc in grp:
                    for a, b in bank_ranges(off, off + RNG[kc]):
                        mm(st[:, a:b], KT[hp][hs, kc * 128:(kc + 1) * 128],
                           QT[hp][hs, 64 * kc + a - off:64 * kc + b - off],
                           start=True, stop=True)
                    offs.append(off)
                    off += RNG[kc]
                pt = pt_pool.tile([128, W], F16, tag="pt", name="pt")
                nc.scalar.activation(pt[:], st[:], A.Exp)
                for kc, off in zip(grp, offs):  # diagonal 0/1 mask
                    nc.vector.tensor_mul(pt[:, off:off + 64],
                                         pt[:, off:off + 64], DM[:])
                if gi == 0 and deferred_norm:
                    deferred_norm.popleft()()
                pend.append((grp, offs, pt, grp is GROUPS[-1]))
                if len(pend) > 2:
                    emit_av(pend.popleft())
                fill_slot()
            while pend:
                emit_av(pend.popleft())
                fill_slot()

            # normalize: evacuate O^T from PSUM immediately (frees the bank
            # for the next head's AV); defer the reciprocal/broadcast/scale
            # past the next head's first mask-muls so they don't block its
            # AV start on the DVE queue. fp16 throughout: numerator/denom
            # magnitudes stay well inside fp16 range and the 2-byte packed
            # SBUF operands hit the DVE 2x/4x fast paths
            ocp = nrm_pool.tile([DH + 1, LOC], F16, tag="ocp", name="ocp")
            nc.vector.tensor_copy(ocp[:], ot[:])

            def norm_rest(ocp=ocp, hp=hp, hs=hs):
                rcp = nrm_pool.tile([1, LOC], F16, tag="rcp", name="rcp")
                nc.vector.reciprocal(rcp[:], ocp[DH:DH + 1, :])
                rbs = nrm_pool.tile([DH, LOC], F16, tag="rbs", name="rbs")
                nc.gpsimd.partition_broadcast(rbs[:], rcp[:])
                nc.vector.tensor_mul(OT[hp][hs, :], ocp[0:DH, :], rbs[:])

            deferred_norm.append(norm_rest)

    while deferred_norm:
        deferred_norm.popleft()()
    while FILL:
        fill(1)

    # ---- out projection + layernorm ----
    scope("outln")
    nrm_pool.release()
    pt_pool.release()
    o_ps.release()
    st_ps.release()
    proj_ps.release()
    xq_pool.release()
    wstr.release()
    xin.release()

    gz = tc.alloc_tile_pool(name="gz", bufs=1)
    grow = gz.tile([1, DIM], F32, tag="grow")
    nc.sync.dma_start(grow[:], lng[:])
    GB = gz.tile([128, DIM], F32, tag="GB")
    nc.gpsimd.partition_broadcast(GB[:], grow[:])
    z_ps = tc.alloc_tile_pool(name="zps", bufs=1, space="PSUM")
    stat = tc.alloc_tile_pool(name="stat", bufs=2)
    stage = tc.alloc_tile_pool(name="stage", bufs=2)

    # bank-panel pipeline: each token-chunk tb runs as two 512-col panel
    # chains; bank-0 stats (sum + sum-of-squares) run on DVE/ACT while the
    # PE fills bank 1, so the post-matmul LN tail of the LAST chunk is just
    # bank-1 stats + combine + normalize instead of a full-row chain
    for tb in range(8):
        zb = []
        ss = []
        qq = []
        for bk in range(2):
            zt = z_ps.tile([128, 512], F32, tag=f"z{(2 * tb + bk) % 8}",
                           name=f"z{tb}_{bk}")
            a = bk * 512
            for hp in range(HP):
                mm(zt[:], OT[hp][:, tb * 128:(tb + 1) * 128],
                   WO[hp][:, a:a + 512], start=(hp == 0), stop=(hp == HP - 1))
            s_ = stat.tile([128, 1], F32, tag=f"s{bk}", name="s")
            nc.vector.reduce_sum(s_[:], zt[:], axis=mybir.AxisListType.X)
            q_ = stat.tile([128, 1], F32, tag=f"q{bk}", name="q")
            scr = stage.tile([128, 512], F32, tag=f"scr{bk}", name="scr")
            nc.scalar.activation(scr[:], zt[:], A.Square, accum_out=q_[:])
            zb.append(zt)
            ss.append(s_)
            qq.append(q_)
        mean_n = stat.tile([128, 1], F32, tag="mean", name="mean")
        nc.vector.tensor_add(mean_n[:], ss[0][:], ss[1][:])
        nc.vector.tensor_scalar_mul(mean_n[:], mean_n[:], -1.0 / DIM)
        msq = stat.tile([128, 1], F32, tag="msq", name="msq")
        nc.vector.tensor_add(msq[:], qq[0][:], qq[1][:])
        nc.vector.tensor_scalar_mul(msq[:], msq[:], 1.0 / DIM)
        var = stat.tile([128, 1], F32, tag="var", name="var")
        nc.vector.tensor_mul(var[:], mean_n[:], mean_n[:])
        nc.vector.tensor_sub(var[:], msq[:], var[:])
        std = stat.tile([128, 1], F32, tag="std", name="std")
        nc.scalar.activation(std[:], var[:], A.Sqrt, bias=epst[:])
        rstd = stat.tile([128, 1], F32, tag="rstd", name="rstd")
        nc.vector.reciprocal(rstd[:], std[:])
        nmr = stat.tile([128, 1], F32, tag="nmr", name="nmr")
        nc.vector.tensor_mul(nmr[:], mean_n[:], rstd[:])
        zn = stage.tile([128, DIM], F32, tag="zn", name="zn")
        outb = stage.tile([128, DIM], F32, tag="outb", name="outb")
        for bk in range(2):  # halves pipeline zn->mul->DMA
            a = bk * 512
            nc.scalar.activation(zn[:, a:a + 512], zb[bk][:], A.Identity,
                                 bias=nmr[:], scale=rstd[:])
            nc.vector.tensor_mul(outb[:, a:a + 512], zn[:, a:a + 512],
                                 GB[:, a:a + 512])
            (nc.sync if tb % 2 == 0 else nc.gpsimd).dma_start(
                out[tb * 128:(tb + 1) * 128, a:a + 512], outb[:, a:a + 512])

    _sc.close()
    stage.release()
    stat.release()
    z_ps.release()
    gz.release()
    resid.release()
    res.release()


def _split8(t):
    """fp32 array -> (hi, lo) e4m3 pair."""
    hi = t.astype(E4)
    lo = (t - hi.astype(np.float32)).astype(E4)
    return hi, lo


def _xpairs(xT):
    """[DIM, n] f32 -> [PR, 128, 2, n] pair layout (chunk 2pr+s rows)."""
    n = xT.shape[1]
    return np.ascontiguousarray(xT.reshape(PR, 2, 128, n).transpose(0, 2, 1, 3))


def _wpairs(w):
    """[DIM, INNER] f32 -> [HP, 128, PR, 2, 128]:
    wb[hp, p, pr, s, j] = w[(2*pr+s)*128 + p, hp*128 + j]."""
    return np.ascontiguousarray(
        w.reshape(PR, 2, 128, HP, 128).transpose(3, 2, 0, 1, 4))


def make_in_maps(x, mask, w_qkv, w_out, ln_g):
    x = np.asarray(x, dtype=np.float32)
    mask_np = np.asarray(mask)
    w_qkv = np.asarray(w_qkv, dtype=np.float32)

    wq_s = w_qkv[:, :INNER] * (SCALE * WQS)
    wk_s = w_qkv[:, INNER:2 * INNER] * WS
    wv_s = w_qkv[:, 2 * INNER:] * WS
    wqh_, wql_ = _split8(wq_s)
    wkh_, wkl_ = _split8(wk_s)
    wvh_, wvl_ = _split8(wv_s)
    wqph = _wpairs(wqh_.astype(np.float32)).astype(E4)
    wqpl = _wpairs(wql_.astype(np.float32)).astype(E4)
    wkph = _wpairs(wkh_.astype(np.float32)).astype(E4)
    wkpl = _wpairs(wkl_.astype(np.float32)).astype(E4)
    # V weights: moving layout [PR, 128, 2, INNER]
    wvph = _xpairs(wvh_.astype(np.float32)).astype(E4)
    wvpl = _xpairs(wvl_.astype(np.float32)).astype(E4)
    wout16 = np.ascontiguousarray(np.asarray(w_out, np.float32)).astype(np.float16)
    lng = np.ascontiguousarray(np.asarray(ln_g, np.float32)[None, :])

    # diagonal 0/1 mask [128 keys, 64 queries]: key 128*kc+kk is causally
    # valid for local query 64*kc+m (global 2*(64*kc+m)+rho) iff kk <= 2m+rho
    kk = np.arange(128)[:, None]
    mq = np.arange(64)[None, :]
    dmasks = {rho: np.ascontiguousarray((kk <= 2 * mq + rho).astype(np.float16))
              for rho in (0, 1)}

    in_maps = []
    for b in range(B):
        xT = np.ascontiguousarray(x[b].T) * XS  # [DIM, N] scaled
        xh_, xl_ = _split8(xT)
        xph = _xpairs(xh_.astype(np.float32)).astype(E4)
        xpl = _xpairs(xl_.astype(np.float32)).astype(E4)
        mv = mask_np[b].astype(np.float32)  # [N]
        mvecT = np.ascontiguousarray(mv.reshape(KC, 128).T) * VDS  # [128, KC]
        mvec1 = np.ascontiguousarray(mv.reshape(KC, 128).T)
        for rho in (0, 1):
            xqT = np.ascontiguousarray(x[b].T[:, rho::2]) * XS
            xqh_, xql_ = _split8(xqT)
            xqh = _xpairs(xqh_.astype(np.float32)).astype(E4)
            xql = _xpairs(xql_.astype(np.float32)).astype(E4)
            in_maps.append({
                "xph": xph, "xpl": xpl, "xqh": xqh, "xql": xql,
                "wkph": wkph, "wkpl": wkpl, "wqph": wqph, "wqpl": wqpl,
                "wvph": wvph, "wvpl": wvpl,
                "wout": wout16, "lng": lng, "dmask": dmasks[rho],
                "mvecT": mvecT, "mvec1": mvec1,
            })
    return in_maps


_CACHE = {}
_LOCK = threading.Lock()
RUN_KWARGS = {}


def _get_nc():
    with _LOCK:
        if "nc" not in _CACHE:
            _CACHE["nc"] = build()
    return _CACHE["nc"]


def kernel(x, mask, w_qkv, w_out, ln_g):
    in_maps = make_in_maps(x, mask, w_qkv, w_out, ln_g)
    nc = _get_nc()
    res = bass_utils.run_bass_kernel_spmd(nc, in_maps, core_ids=list(range(NC)),
                                          **RUN_KWARGS)
    _CACHE["last"] = res

    final = np.empty((B, N, DIM), dtype=np.float32)
    for b in range(B):
        for rho in (0, 1):
            final[b, rho::2, :] = res.results[2 * b + rho]["out"]
    return final


# revision 8
# speedup vs baseline: 1.0428x; 1.0046x over previous
"""CLIP attention block (LN(attn(x) @ W_out)) on 8 TRN2 NeuronCores. v3.

Problem (hardcoded): x [4, 2048, 1024] f32, mask [4, 2048] bool,
w_qkv [1024, 3072], w_out [1024, 1024], ln_g [1024].
16 heads x 64 dim, causal, scale = 1/8. Output [4, 2048, 1024] f32.

Sharding: core = (batch b, parity rho). Each core computes the final
output rows for queries of batch b with index % 2 == rho (1024 tokens);
interleaving by parity makes the causal workload identical on every core.

v3 vs v2 (331.4us): Q/K/V projections switched from fp16 matmuls to
fp8(e4m3) DoubleRow 3-term compensated matmuls: x and w are host-split
into (hi, lo) e4m3 pairs (hi = e4m3(s*t), lo = e4m3(s*t - hi)) and each
128-contraction chunk pair computes hi*hi + lo*hi + hi*lo (dropping the
~0.07% lo*lo term). DoubleRow packs 2 chunks per instruction at 0.5
cycles/row, so 8 fp16 chunk-matmuls become 12 DR matmuls at 0.75x the
PE rows with fp16-grade accuracy (e2e rel err ~2e-3 vs gate 2e-2).
Scales: x*16, w_k/w_v*32, w_q*256 (attention scale folded); descaling
is folded into the existing PSUM evacuations (1/512 into the V mask
multiplier host-side, 1/512 and 1/4096 into the K/Q copies).
fp16 attention (scores, exp, AV) and out-proj/LN unchanged from v2:
- Exact causal ranges + diagonal 0/1 mask; denominator via V aug column.
- Exp batched into 9 Act instructions/head; software-pipelined AV.
- Remaining K/Q/V projection chunks run as deadline-scheduled filler
  chains inside attention.
HW pitfalls hit: gpsimd cannot access PSUM; matmul outputs cannot cross
a PSUM bank boundary; in-place DVE reciprocal corrupts on HW (fine in
CoreSim) - use a separate output tile.
"""

import threading
from collections import deque
from contextlib import ExitStack

import numpy as np
import ml_dtypes

import concourse.bass as bass
import concourse.mybir as mybir
import concourse.tile as tile
from concourse import bacc
import concourse.bass_utils as bass_utils

F8 = mybir.dt.float8e4
F16 = mybir.dt.float16
F32 = mybir.dt.float32
E4 = ml_dtypes.float8_e4m3
DRMODE = mybir.MatmulPerfMode.DoubleRow

B, N, DIM = 4, 2048, 1024
HEADS, DH = 16, 64
INNER = HEADS * DH          # 1024
SCALE = DH ** -0.5          # 0.125
LOC = N // 2                # 1024 local query tokens per core
EPS = 1e-5

NC = 8                      # cores
HP = HEADS // 2             # 8 head pairs (128 inner dims each)
PR = 4                      # dim-chunk pairs (contraction 1024 = 4 x 256)
KC = N // 128               # 16 key chunks

XS = 16.0                   # fp8 quantization scale for x
WS = 32.0                   # fp8 scale for w_k / w_v
WQS = 256.0                 # fp8 scale for w_q (SCALE folded -> smaller w)
KDS = 1.0 / (XS * WS)       # kproj descale (psum -> KT)
QDS = 1.0 / (XS * WQS)      # qproj descale
VDS = 1.0 / (XS * WS)       # vproj descale (folded into MV host-side)

# q-range of key chunk kc: local queries [64*kc, LOC)
RNG = [LOC - 64 * kc for kc in range(KC)]


def bank_ranges(c0, c1):
    """Split [c0, c1) at 512-column boundaries (PSUM f32 bank size) —
    a matmul output may not cross a PSUM bank boundary."""
    a = c0
    while a < c1:
        b = min(c1, (a // 512 + 1) * 512)
        yield a, b
        a = b
# exp batching: groups of key chunks with total scores width <= 1024
# (kc j paired with 16-j so every pair is exactly 1024 wide)
GROUPS = [[0], [1, 15], [2, 14], [3, 13], [4, 12], [5, 11], [6, 10],
          [7, 9], [8]]


def build(reps=1):
    nc = bacc.Bacc("TRN2", target_bir_lowering=False, debug=False, num_devices=NC)

    # x^T as dim-chunk-pair tiles, e4m3 hi/lo: [pr][128][slot 2][token]
    xph = nc.dram_tensor("xph", [PR, 128, 2, N], F8, kind="ExternalInput").ap()
    xpl = nc.dram_tensor("xpl", [PR, 128, 2, N], F8, kind="ExternalInput").ap()
    xqh = nc.dram_tensor("xqh", [PR, 128, 2, LOC], F8, kind="ExternalInput").ap()
    xql = nc.dram_tensor("xql", [PR, 128, 2, LOC], F8, kind="ExternalInput").ap()
    # K/Q weights as stationary pair chunks: [hp][128][pr][slot 2][inner 128]
    wkph = nc.dram_tensor("wkph", [HP, 128, PR, 2, 128], F8, kind="ExternalInput").ap()
    wkpl = nc.dram_tensor("wkpl", [HP, 128, PR, 2, 128], F8, kind="ExternalInput").ap()
    wqph = nc.dram_tensor("wqph", [HP, 128, PR, 2, 128], F8, kind="ExternalInput").ap()
    wqpl = nc.dram_tensor("wqpl", [HP, 128, PR, 2, 128], F8, kind="ExternalInput").ap()
    # V weights as moving pair tiles: [pr][128][slot 2][inner]
    wvph = nc.dram_tensor("wvph", [PR, 128, 2, INNER], F8, kind="ExternalInput").ap()
    wvpl = nc.dram_tensor("wvpl", [PR, 128, 2, INNER], F8, kind="ExternalInput").ap()
    wout = nc.dram_tensor("wout", [INNER, DIM], F16, kind="ExternalInput").ap()
    lng = nc.dram_tensor("lng", [1, DIM], F32, kind="ExternalInput").ap()
    dmask = nc.dram_tensor("dmask", [128, 64], F16, kind="ExternalInput").ap()
    # mvecT = mask * VDS (V descale folded); mvec1 = raw mask for aug col
    mvecT = nc.dram_tensor("mvecT", [128, KC], F32, kind="ExternalInput").ap()
    mvec1 = nc.dram_tensor("mvec1", [128, KC], F32, kind="ExternalInput").ap()
    out = nc.dram_tensor("out", [LOC, DIM], F32, kind="ExternalOutput").ap()

    with nc.allow_low_precision(reason="fp8/fp16 matmul staging"), \
         tile.TileContext(nc) as tc:
        for _ in range(reps):
            _build_body(nc, tc, xph, xpl, xqh, xql, wkph, wkpl, wqph, wqpl,
                        wvph, wvpl, wout, lng, dmask, mvecT, mvec1, out)

    nc.compile()
    return nc


def _build_body(nc, tc, xph, xpl, xqh, xql, wkph, wkpl, wqph, wqpl,
                wvph, wvpl, wout, lng, dmask, mvecT, mvec1, out):
    mm = nc.tensor.matmul
    A = mybir.ActivationFunctionType
    _sc = ExitStack()

    def scope(name):
        _sc.close()
        _sc.enter_context(nc.named_scope(name))

    scope("prep")
    res = tc.alloc_tile_pool(name="res", bufs=1)
    dzero = res.tile([128, 640], F16, tag="dzero")
    nc.vector.memset(dzero[:], 0.0)
    DM = res.tile([128, 64], F16, tag="DM")
    nc.sync.dma_start(DM[:], dmask[:])
    MV = res.tile([128, KC], F32, tag="MV")
    nc.sync.dma_start(MV[:], mvecT[:])
    MV1 = res.tile([128, KC], F32, tag="MV1")
    nc.sync.dma_start(MV1[:], mvec1[:])
    ones16 = res.tile([128, HEADS, 1], F32, tag="ones16")
    nc.vector.memset(ones16[:], 1.0)
    epst = res.tile([128, 1], F32, tag="epst")
    nc.vector.memset(epst[:], EPS)

    # ---- long-lived residents (alloc order = reverse release order) ----
    resid = tc.alloc_tile_pool(name="resid", bufs=1)
    V = [resid.tile([128, HEADS, DH + 1], F16, tag=f"V{i}", name=f"V{i}")
         for i in range(KC)]
    KT = [resid.tile([128, N], F16, tag=f"kt{hp}", name=f"kt{hp}")
          for hp in range(HP)]
    QT = [resid.tile([128, LOC], F16, tag=f"qt{hp}", name=f"qt{hp}")
          for hp in range(HP)]
    OT = [resid.tile([128, LOC], F16, tag=f"otl{hp}", name=f"otl{hp}")
          for hp in range(HP)]
    WO = [resid.tile([128, DIM], F16, tag=f"wo{hp}", name=f"wo{hp}")
          for hp in range(HP)]

    # ---- input slabs (e4m3 hi/lo pair tiles) ----
    xin = tc.alloc_tile_pool(name="xin", bufs=1)
    XFH = [xin.tile([128, 2, N], F8, tag=f"xfh{pr}", name=f"xfh{pr}")
           for pr in range(PR)]
    XFL = [xin.tile([128, 2, N], F8, tag=f"xfl{pr}", name=f"xfl{pr}")
           for pr in range(PR)]
    WVH = [xin.tile([128, 2, INNER], F8, tag=f"wvh{pr}", name=f"wvh{pr}")
           for pr in range(PR)]
    WVL = [xin.tile([128, 2, INNER], F8, tag=f"wvl{pr}", name=f"wvl{pr}")
           for pr in range(PR)]
    wstr = tc.alloc_tile_pool(name="wstr", bufs=10)
    xq_pool = tc.alloc_tile_pool(name="xq", bufs=1)
    XQH = [xq_pool.tile([128, 2, LOC], F8, tag=f"xqh{pr}", name=f"xqh{pr}")
           for pr in range(PR)]
    XQL = [xq_pool.tile([128, 2, LOC], F8, tag=f"xql{pr}", name=f"xql{pr}")
           for pr in range(PR)]

    WKC = {}

    def wkc_load(hp):
        wkch = wstr.tile([128, PR, 2, 128], F8, tag="wkc", name="wkch")
        nc.gpsimd.dma_start(wkch[:], wkph[hp])
        wkcl = wstr.tile([128, PR, 2, 128], F8, tag="wkc", name="wkcl")
        nc.gpsimd.dma_start(wkcl[:], wkpl[hp])
        WKC[hp] = (wkch, wkcl)
    # V-projection inputs first (halved transfers so vproj starts early):
    # sync: XFH halves; scalar: XFL, then XQ, then WV second halves;
    # gpsimd: WV first halves, then streamed K/Q weight chunks.
    for q0, q1 in ((0, 512), (512, 1024)):
        for pr in range(PR):
            nc.sync.dma_start(XFH[pr][:, :, q0:q1], xph[pr][:, :, q0:q1])
            nc.scalar.dma_start(XFL[pr][:, :, q0:q1], xpl[pr][:, :, q0:q1])
            if q0 == 0:
                nc.gpsimd.dma_start(WVH[pr][:, :, 0:512],
                                    wvph[pr][:, :, 0:512])
                nc.gpsimd.dma_start(WVL[pr][:, :, 0:512],
                                    wvpl[pr][:, :, 0:512])
    wkc_load(0)  # hp0 K-weights right behind the WV first halves
    for pr in range(PR):
        nc.sync.dma_start(XFH[pr][:, :, N // 2:], xph[pr][:, :, N // 2:])
        nc.scalar.dma_start(XFL[pr][:, :, N // 2:], xpl[pr][:, :, N // 2:])
        nc.scalar.dma_start(XQH[pr][:], xqh[pr])
        nc.scalar.dma_start(XQL[pr][:], xql[pr])
        nc.sync.dma_start(WVH[pr][:, :, 512:], wvph[pr][:, :, 512:])
        nc.sync.dma_start(WVL[pr][:, :, 512:], wvpl[pr][:, :, 512:])

    # 3-term fp8 step list for one chunk pair: hi*hi + lo*hi + hi*lo
    def dr_steps(hi_s, lo_s, hi_m, lo_m):
        return ((hi_s, hi_m), (lo_s, hi_m), (hi_s, lo_m))

    # ---- projection chunk emitters ----
    def vproj_chunk(tci, ig, ps_pool):
        vpt = ps_pool.tile([128, 1024], F32, tag="pp", name="vp")
        vp = vpt[:, 0:512]
        ts = slice(tci * 128, (tci + 1) * 128)
        ws = slice(ig * 512, (ig + 1) * 512)
        for pr in range(PR):
            for si, (s, m) in enumerate(dr_steps(XFH[pr], XFL[pr],
                                                 WVH[pr], WVL[pr])):
                mm(vp, s[:, :, ts], m[:, :, ws],
                   start=(pr == 0 and si == 0), stop=(pr == 3 and si == 2),
                   perf_mode=DRMODE)
        nc.vector.tensor_scalar_mul(
            V[tci][:, ig * 8:(ig + 1) * 8, 0:DH],
            vp.rearrange("p (h d) -> p h d", d=DH), MV[:, tci:tci + 1])
        nc.vector.tensor_scalar_mul(
            V[tci][:, ig * 8:(ig + 1) * 8, DH:DH + 1],
            ones16[:, ig * 8:(ig + 1) * 8, :], MV1[:, tci:tci + 1])

    def kproj_chunk(hp, tg, ps_pool, copy_eng):
        kp = ps_pool.tile([128, 1024], F32, tag="pp", name="kp")
        wh, wl = WKC[hp]
        for pr in range(PR):
            for si, (s, m) in enumerate(dr_steps(wh[:, pr], wl[:, pr],
                                                 XFH[pr], XFL[pr])):
                for a, b in bank_ranges(0, 1024):
                    mm(kp[:, a:b], s, m[:, :, tg * 1024 + a:tg * 1024 + b],
                       start=(pr == 0 and si == 0),
                       stop=(pr == 3 and si == 2),
                       perf_mode=DRMODE)
        dst = KT[hp][:, tg * 1024:(tg + 1) * 1024]
        if copy_eng == "act":
            nc.scalar.activation(dst, kp[:], A.Identity, scale=KDS)
        else:
            nc.vector.tensor_scalar_mul(dst, kp[:], KDS)

    def qproj_chunk(hp, wqc, ps_pool, copy_eng):
        qp = ps_pool.tile([128, 1024], F32, tag="pp", name="qp")
        wh, wl = wqc
        for pr in range(PR):
            for si, (s, m) in enumerate(dr_steps(wh[:, pr], wl[:, pr],
                                                 XQH[pr], XQL[pr])):
                for a, b in bank_ranges(0, 1024):
                    mm(qp[:, a:b], s, m[:, :, a:b],
                       start=(pr == 0 and si == 0),
                       stop=(pr == 3 and si == 2),
                       perf_mode=DRMODE)
        if copy_eng == "act":
            nc.scalar.activation(QT[hp][:], qp[:], A.Identity, scale=QDS)
        else:
            nc.vector.tensor_scalar_mul(QT[hp][:], qp[:], QDS)

    # ---- pre-attention: vproj heads 0-7, all qproj, kproj hp0 ----
    # vproj ig=0 in two pr-major waves of 8 chains so the PE starts on
    # partial inputs and stays fed while XF/WV chunks stream in
    wave_ps = tc.alloc_tile_pool(name="waveps", bufs=8, space="PSUM")
    scope("vproj")
    # warm-up matmuls on zeros: keep the PE busy through the first input
    # DMA latency so the clock is fully ramped (p-state) when vproj starts
    dum = wave_ps.tile([128, 512], F32, tag="vp", name="dum")
    for _ in range(10):
        mm(dum[:], dzero[:, 0:128], dzero[:, 128:640],
           start=True, stop=True)
    for wv_ in range(4):
        tcis = range(wv_ * 4, wv_ * 4 + 4)
        vps = {tci: wave_ps.tile([128, 512], F32, tag="vp", name=f"vp{tci}")
               for tci in tcis}
        for pr in range(PR):
            for si, (s, m) in enumerate(dr_steps(XFH[pr], XFL[pr],
                                                 WVH[pr], WVL[pr])):
                for tci in tcis:
                    mm(vps[tci][:], s[:, :, tci * 128:(tci + 1) * 128],
                       m[:, :, 0:512],
                       start=(pr == 0 and si == 0),
                       stop=(pr == 3 and si == 2), perf_mode=DRMODE)
        for tci in tcis:
            nc.vector.tensor_scalar_mul(
                V[tci][:, 0:8, 0:DH],
                vps[tci][:].rearrange("p (h d) -> p h d", d=DH),
                MV[:, tci:tci + 1])
            nc.vector.tensor_scalar_mul(
                V[tci][:, 0:8, DH:DH + 1], ones16[:, 0:8, :],
                MV1[:, tci:tci + 1])
    # kproj0 tg0 uses the still-live wave pool (two 1-bank tiles) so it
    # doesn't wait on the wave->preps pool release barrier
    scope("kproj0")
    kpA = wave_ps.tile([128, 512], F32, tag="vp", name="kpA")
    kpB = wave_ps.tile([128, 512], F32, tag="vp", name="kpB")
    wh0, wl0 = WKC[0]
    for pr in range(PR):
        for si, (s, m) in enumerate(dr_steps(wh0[:, pr], wl0[:, pr],
                                             XFH[pr], XFL[pr])):
            mm(kpA[:], s, m[:, :, 0:512],
               start=(pr == 0 and si == 0), stop=(pr == 3 and si == 2),
               perf_mode=DRMODE)
            mm(kpB[:], s, m[:, :, 512:1024],
               start=(pr == 0 and si == 0), stop=(pr == 3 and si == 2),
               perf_mode=DRMODE)
    nc.scalar.activation(KT[0][:, 0:512], kpA[:], A.Identity, scale=KDS)
    nc.scalar.activation(KT[0][:, 512:1024], kpB[:], A.Identity, scale=KDS)
    scope("qproj")
    wqcs = {}

    def wqc_load(pf):
        wqch = wstr.tile([128, PR, 2, 128], F8, tag="wkc", name="wqch")
        nc.gpsimd.dma_start(wqch[:], wqph[pf])
        wqcl = wstr.tile([128, PR, 2, 128], F8, tag="wkc", name="wqcl")
        nc.gpsimd.dma_start(wqcl[:], wqpl[pf])
        wqcs[pf] = (wqch, wqcl)

    for pf in (0, 1, 2):
        wqc_load(pf)
    qpA = wave_ps.tile([128, 512], F32, tag="vp", name="qpA")
    qpB = wave_ps.tile([128, 512], F32, tag="vp", name="qpB")
    whq, wlq = wqcs.pop(0)
    for pr in range(PR):
        for si, (st_, mv_) in enumerate(dr_steps(whq[:, pr], wlq[:, pr],
                                                 XQH[pr], XQL[pr])):
            mm(qpA[:], st_, mv_[:, :, 0:512],
               start=(pr == 0 and si == 0), stop=(pr == 3 and si == 2),
               perf_mode=DRMODE)
            mm(qpB[:], st_, mv_[:, :, 512:1024],
               start=(pr == 0 and si == 0), stop=(pr == 3 and si == 2),
               perf_mode=DRMODE)
    nc.scalar.activation(QT[0][:, 0:512], qpA[:], A.Identity, scale=QDS)
    nc.scalar.activation(QT[0][:, 512:1024], qpB[:], A.Identity, scale=QDS)
    scope("kproj0b")
    kpC = wave_ps.tile([128, 512], F32, tag="vp", name="kpC")
    kpD = wave_ps.tile([128, 512], F32, tag="vp", name="kpD")
    for pr in range(PR):
        for si, (st_, mv_) in enumerate(dr_steps(wh0[:, pr], wl0[:, pr],
                                                 XFH[pr], XFL[pr])):
            mm(kpC[:], st_, mv_[:, :, 1024:1536],
               start=(pr == 0 and si == 0), stop=(pr == 3 and si == 2),
               perf_mode=DRMODE)
            mm(kpD[:], st_, mv_[:, :, 1536:2048],
               start=(pr == 0 and si == 0), stop=(pr == 3 and si == 2),
               perf_mode=DRMODE)
    nc.scalar.activation(KT[0][:, 1024:1536], kpC[:], A.Identity, scale=KDS)
    nc.scalar.activation(KT[0][:, 1536:2048], kpD[:], A.Identity, scale=KDS)

    for hp in range(HP):
        (nc.sync if hp % 2 == 0 else nc.scalar).dma_start(
            WO[hp][:], wout[hp * 128:(hp + 1) * 128, :])

    # ---- attention (+ interleaved remaining projections) ----
    wave_ps.release()
    proj_ps = tc.alloc_tile_pool(name="projps", bufs=1, space="PSUM")
    st_ps = tc.alloc_tile_pool(name="stps", bufs=2, space="PSUM")
    o_ps = tc.alloc_tile_pool(name="ops", bufs=1, space="PSUM")
    pt_pool = tc.alloc_tile_pool(name="pt", bufs=3)
    nrm_pool = tc.alloc_tile_pool(name="nrm", bufs=1)

    # filler chains, deadline-interleaved: kproj(hp) must land well before
    # attention head 2*hp; vproj heads 8-15 before head 8. One chain fires
    # every 3rd pipeline slot (9 slots per head).
    wkc_load(1)

    def kf(hp, tg):
        def f():
            kproj_chunk(hp, tg, proj_ps, "dve")
            if tg == 1:
                if hp + 1 < HP:
                    wkc_load(hp + 1)
                WKC.pop(hp)
        return f

    def vf(tci):
        return lambda: vproj_chunk(tci, 1, proj_ps)

    def qf(hp):
        def f():
            if hp + 2 < HP:
                wqc_load(hp + 2)
            qproj_chunk(hp, wqcs.pop(hp), proj_ps, "dve")
        return f

    FILL = deque([
        qf(1), kf(1, 0), kf(1, 1), vf(0), vf(1),
        qf(2), kf(2, 0), kf(2, 1), vf(2), vf(3),
        qf(3), kf(3, 0), kf(3, 1), vf(4), vf(5),
        qf(4), kf(4, 0), kf(4, 1), vf(6), vf(7),
        vf(8), vf(9), vf(10), vf(11), vf(12), vf(13), vf(14), vf(15),
        qf(5), kf(5, 0), kf(5, 1), qf(6), kf(6, 0), kf(6, 1),
        qf(7), kf(7, 0), kf(7, 1),
    ])

    def fill(n=1):
        for _ in range(n):
            if FILL:
                with nc.named_scope("fillp"):
                    FILL.popleft()()

    _slot = [0, 2, 0]

    def fill_slot():
        # spread filler chains over the attention span: fast enough early
        # that every chain beats its consumer head (~1 per 2.5 slots),
        # stretched to 1 per 5 for the last chains so late Act-bound heads
        # still have PE work
        _slot[0] += 1
        if _slot[0] >= _slot[1]:
            fill(1)
            if len(FILL) > 9:
                _slot[2] ^= 1
                _slot[1] += 2 + _slot[2]
            else:
                _slot[1] += 5

    scope("attn")
    deferred_norm = deque()
    for hp in range(HP):
        for h2 in range(2):
            h = 2 * hp + h2
            hs = slice(h2 * DH, (h2 + 1) * DH)
            ot = o_ps.tile([DH + 1, LOC], F32, tag="o", name="ot")

            def emit_av(p, ot=ot, h=h):
                grp, offs, pt, last = p
                for gi, (kc, off) in enumerate(zip(grp, offs)):
                    pieces = list(bank_ranges(64 * kc, LOC))
                    for pi, (a, b) in enumerate(pieces):
                        mm(ot[:, a:b], V[kc][:, h, :],
                           pt[:, off + a - 64 * kc:off + b - 64 * kc],
                           start=(kc == 0),
                           stop=(last and gi == len(grp) - 1
                                 and pi == len(pieces) - 1),
                           skip_group_check=True)

            pend = deque()  # software pipeline, depth 2
            for gi, grp in enumerate(GROUPS):
                W = sum(RNG[kc] for kc in grp)
                st = st_ps.tile([128, W], F32, tag="st", name="st")
                offs = []
                off = 0
                for kc in grp:
                    for a, b in bank_ranges(off, off + RNG[kc]):
                        mm(st[:, a:b], KT[hp][hs, kc * 128:(kc + 1) * 128],
                           QT[hp][hs, 64 * kc + a - off:64 * kc + b - off],
                           start=True, stop=True)
                    offs.append(off)
                    off += RNG[kc]
                pt = pt_pool.tile([128, W], F16, tag="pt", name="pt")
                nc.scalar.activation(pt[:], st[:], A.Exp)
                for kc, off in zip(grp, offs):  # diagonal 0/1 mask
                    nc.vector.tensor_mul(pt[:, off:off + 64],
                                         pt[:, off:off + 64], DM[:])
                if gi == 0 and deferred_norm:
                    deferred_norm.popleft()()
                pend.append((grp, offs, pt, grp is GROUPS[-1]))
                if len(pend) > 2:
                    emit_av(pend.popleft())
                fill_slot()
            while pend:
                emit_av(pend.popleft())
                fill_slot()

            # normalize: evacuate O^T from PSUM immediately (frees the bank
            # for the next head's AV); defer the reciprocal/broadcast/scale
            # past the next head's first mask-muls so they don't block its
            # AV start on the DVE queue. fp16 throughout: numerator/denom
            # magnitudes stay well inside fp16 range and the 2-byte packed
            # SBUF operands hit the DVE 2x/4x fast paths
            ocp = nrm_pool.tile([DH + 1, LOC], F16, tag="ocp", name="ocp")
            nc.vector.tensor_copy(ocp[:], ot[:])

            def norm_rest(ocp=ocp, hp=hp, hs=hs):
                rcp = nrm_pool.tile([1, LOC], F16, tag="rcp", name="rcp")
                nc.vector.reciprocal(rcp[:], ocp[DH:DH + 1, :])
                rbs = nrm_pool.tile([DH, LOC], F16, tag="rbs", name="rbs")
                nc.gpsimd.partition_broadcast(rbs[:], rcp[:])
                nc.vector.tensor_mul(OT[hp][hs, :], ocp[0:DH, :], rbs[:])

            deferred_norm.append(norm_rest)

    while deferred_norm:
        deferred_norm.popleft()()
    while FILL:
        fill(1)

    # ---- out projection + layernorm ----
    scope("outln")
    nrm_pool.release()
    pt_pool.release()
    o_ps.release()
    st_ps.release()
    proj_ps.release()
    xq_pool.release()
    wstr.release()
    xin.release()

    gz = tc.alloc_tile_pool(name="gz", bufs=1)
    grow = gz.tile([1, DIM], F32, tag="grow")
    nc.sync.dma_start(grow[:], lng[:])
    GB = gz.tile([128, DIM], F32, tag="GB")
    nc.gpsimd.partition_broadcast(GB[:], grow[:])
    z_ps = tc.alloc_tile_pool(name="zps", bufs=1, space="PSUM")
    stat = tc.alloc_tile_pool(name="stat", bufs=2)
    stage = tc.alloc_tile_pool(name="stage", bufs=2)

    # bank-panel pipeline: each token-chunk tb runs as two 512-col panel
    # chains; bank-0 stats (sum + sum-of-squares) run on DVE/ACT while the
    # PE fills bank 1, so the post-matmul LN tail of the LAST chunk is just
    # bank-1 stats + combine + normalize instead of a full-row chain
    for tb in range(8):
        zb = []
        ss = []
        qq = []
        for bk in range(2):
            zt = z_ps.tile([128, 512], F32, tag=f"z{(2 * tb + bk) % 8}",
                           name=f"z{tb}_{bk}")
            a = bk * 512
            for hp in range(HP):
                mm(zt[:], OT[hp][:, tb * 128:(tb + 1) * 128],
                   WO[hp][:, a:a + 512], start=(hp == 0), stop=(hp == HP - 1))
            s_ = stat.tile([128, 1], F32, tag=f"s{bk}", name="s")
            nc.vector.reduce_sum(s_[:], zt[:], axis=mybir.AxisListType.X)
            q_ = stat.tile([128, 1], F32, tag=f"q{bk}", name="q")
            scr = stage.tile([128, 512], F32, tag=f"scr{bk}", name="scr")
            nc.scalar.activation(scr[:], zt[:], A.Square, accum_out=q_[:])
            zb.append(zt)
            ss.append(s_)
            qq.append(q_)
        mean_n = stat.tile([128, 1], F32, tag="mean", name="mean")
        nc.vector.tensor_add(mean_n[:], ss[0][:], ss[1][:])
        nc.vector.tensor_scalar_mul(mean_n[:], mean_n[:], -1.0 / DIM)
        msq = stat.tile([128, 1], F32, tag="msq", name="msq")
        nc.vector.tensor_add(msq[:], qq[0][:], qq[1][:])
        nc.vector.tensor_scalar_mul(msq[:], msq[:], 1.0 / DIM)
        var = stat.tile([128, 1], F32, tag="var", name="var")
        nc.vector.tensor_mul(var[:], mean_n[:], mean_n[:])
        nc.vector.tensor_sub(var[:], msq[:], var[:])
        std = stat.tile([128, 1], F32, tag="std", name="std")
        nc.scalar.activation(std[:], var[:], A.Sqrt, bias=epst[:])
        rstd = stat.tile([128, 1], F32, tag="rstd", name="rstd")
        nc.vector.reciprocal(rstd[:], std[:])
        nmr = stat.tile([128, 1], F32, tag="nmr", name="nmr")
        nc.vector.tensor_mul(nmr[:], mean_n[:], rstd[:])
        zn = stage.tile([128, DIM], F32, tag="zn", name="zn")
        outb = stage.tile([128, DIM], F32, tag="outb", name="outb")
        for bk in range(2):  # halves pipeline zn->mul->DMA
            a = bk * 512
            nc.scalar.activation(zn[:, a:a + 512], zb[bk][:], A.Identity,
                                 bias=nmr[:], scale=rstd[:])
            nc.vector.tensor_mul(outb[:, a:a + 512], zn[:, a:a + 512],
                                 GB[:, a:a + 512])
            (nc.sync if tb % 2 == 0 else nc.gpsimd).dma_start(
                out[tb * 128:(tb + 1) * 128, a:a + 512], outb[:, a:a + 512])

    _sc.close()
    stage.release()
    stat.release()
    z_ps.release()
    gz.release()
    resid.release()
    res.release()


def _split8(t):
    """fp32 array -> (hi, lo) e4m3 pair."""
    hi = t.astype(E4)
    lo = (t - hi.astype(np.float32)).astype(E4)
    return hi, lo


def _xpairs(xT):
    """[DIM, n] f32 -> [PR, 128, 2, n] pair layout (chunk 2pr+s rows)."""
    n = xT.shape[1]
    return np.ascontiguousarray(xT.reshape(PR, 2, 128, n).transpose(0, 2, 1, 3))


def _wpairs(w):
    """[DIM, INNER] f32 -> [HP, 128, PR, 2, 128]:
    wb[hp, p, pr, s, j] = w[(2*pr+s)*128 + p, hp*128 + j]."""
    return np.ascontiguousarray(
        w.reshape(PR, 2, 128, HP, 128).transpose(3, 2, 0, 1, 4))


def make_in_maps(x, mask, w_qkv, w_out, ln_g):
    x = np.asarray(x, dtype=np.float32)
    mask_np = np.asarray(mask)
    w_qkv = np.asarray(w_qkv, dtype=np.float32)

    wq_s = w_qkv[:, :INNER] * (SCALE * WQS)
    wk_s = w_qkv[:, INNER:2 * INNER] * WS
    wv_s = w_qkv[:, 2 * INNER:] * WS
    wqh_, wql_ = _split8(wq_s)
    wkh_, wkl_ = _split8(wk_s)
    wvh_, wvl_ = _split8(wv_s)
    wqph = _wpairs(wqh_.astype(np.float32)).astype(E4)
    wqpl = _wpairs(wql_.astype(np.float32)).astype(E4)
    wkph = _wpairs(wkh_.astype(np.float32)).astype(E4)
    wkpl = _wpairs(wkl_.astype(np.float32)).astype(E4)
    # V weights: moving layout [PR, 128, 2, INNER]
    wvph = _xpairs(wvh_.astype(np.float32)).astype(E4)
    wvpl = _xpairs(wvl_.astype(np.float32)).astype(E4)
    wout16 = np.ascontiguousarray(np.asarray(w_out, np.float32)).astype(np.float16)
    lng = np.ascontiguousarray(np.asarray(ln_g, np.float32)[None, :])

    # diagonal 0/1 mask [128 keys, 64 queries]: key 128*kc+kk is causally
    # valid for local query 64*kc+m (global 2*(64*kc+m)+rho) iff kk <= 2m+rho
    kk = np.arange(128)[:, None]
    mq = np.arange(64)[None, :]
    dmasks = {rho: np.ascontiguousarray((kk <= 2 * mq + rho).astype(np.float16))
              for rho in (0, 1)}

    in_maps = []
    for b in range(B):
        xT = np.ascontiguousarray(x[b].T) * XS  # [DIM, N] scaled
        xh_, xl_ = _split8(xT)
        xph = _xpairs(xh_.astype(np.float32)).astype(E4)
        xpl = _xpairs(xl_.astype(np.float32)).astype(E4)
        mv = mask_np[b].astype(np.float32)  # [N]
        mvecT = np.ascontiguousarray(mv.reshape(KC, 128).T) * VDS  # [128, KC]
        mvec1 = np.ascontiguousarray(mv.reshape(KC, 128).T)
        for rho in (0, 1):
            xqT = np.ascontiguousarray(x[b].T[:, rho::2]) * XS
            xqh_, xql_ = _split8(xqT)
            xqh = _xpairs(xqh_.astype(np.float32)).astype(E4)
            xql = _xpairs(xql_.astype(np.float32)).astype(E4)
            in_maps.append({
                "xph": xph, "xpl": xpl, "xqh": xqh, "xql": xql,
                "wkph": wkph, "wkpl": wkpl, "wqph": wqph, "wqpl": wqpl,
                "wvph": wvph, "wvpl": wvpl,
                "wout": wout16, "lng": lng, "dmask": dmasks[rho],
                "mvecT": mvecT, "mvec1": mvec1,
            })
    return in_maps


_CACHE = {}
_LOCK = threading.Lock()
RUN_KWARGS = {}


def _get_nc():
    with _LOCK:
        if "nc" not in _CACHE:
            _CACHE["nc"] = build()
    return _CACHE["nc"]


def kernel(x, mask, w_qkv, w_out, ln_g):
    in_maps = make_in_maps(x, mask, w_qkv, w_out, ln_g)
    nc = _get_nc()
    res = bass_utils.run_bass_kernel_spmd(nc, in_maps, core_ids=list(range(NC)),
                                          **RUN_KWARGS)
    _CACHE["last"] = res

    final = np.empty((B, N, DIM), dtype=np.float32)
    for b in range(B):
        for rho in (0, 1):
            final[b, rho::2, :] = res.results[2 * b + rho]["out"]
    return final
